# revision 6
# baseline (speedup 1.0000x reference)
"""Trainium2 Bass kernel for the controlled-U (CU) gate application.

Math: the reference builds U = P0 (x) I (x) ... + P1 (x) Mexp (x) I ...
with dim=2, wires=12, index=(0,1), control_state=(1,). This factors as

    U = diag(I_2048, Mexp (x) I_1024)        (4096 x 4096)

so U @ x is:
    out[0:2048]     = x[0:2048]                        (identity)
    out[2048:3072]  = c00 * x[2048:3072] + c01 * x[3072:4096]
    out[3072:4096]  = c10 * x[2048:3072] + c11 * x[3072:4096]

with [[c00, c01], [c10, c11]] = Mexp = expm(M - M^H), a 2x2 unitary
computed exactly on host (eigendecomposition of the 2x2 Hermitian
generator).

The identity block is a no-op: the top 2048 output rows are assembled on
the host directly from x (exact, f32). Only the bottom 2048 rows -- the
actual 2x2 complex mix -- run on the device.

Device strategy (8 NeuronCores, SPMD, fp16):
  - core d owns pair-rows r in [128d, 128d+128): block-1 row 2048+r and
    block-2 row 3072+r.
  - the host packs, per core, an fp16 tile xb[128, 128+4096]: cols 0:128
    hold the stationary W = kron(C^T, I_32) (so W rides the same 2 KiB-
    row load stream instead of straggling as a 256 B-packet transfer);
    column group g in 0..3 holds pair-rows 32g..32g+31 at cols
    128+1024g, partitions stacked [x1_re(32); x1_im(32); x2_re(32);
    x2_im(32)].
  - one 128x128 fp16 stationary turns each matmul into all four output
    quantities at once: out partition q*32+r is quantity q (o1re, o1im,
    o2re, o2im) of pair-row r.  8 matmuls of [128, 512] cover the batch.
  - dummy matmuls on a zeroed scratch warm the PE HAM clock gate during
    the load window so payload matmuls run at 2.4 GHz, not 1.2.
  - PSUM f32 -> SBUF f16 drain alternates DVE / ACT so the cast stream
    is not serialized on one engine; stores ride the sync HWDGE ring
    behind the loads.
  - the host unpacks the fp16 quantities into the complex64 result.

fp16 quantization of inputs/outputs gives ~5e-4 relative error, far
inside the 2e-2 gate; the top half is exact.
"""

import numpy as np

import concourse.bacc as bacc
import concourse.mybir as mybir
from concourse.tile import TileContext
from concourse.bass_utils import run_bass_kernel_spmd

# Problem geometry (hardcoded per the task contract).
D = 4096           # state dimension 2**12
B = 1024           # batch
NCORES = 8
P = 128            # SBUF partitions
G = 4              # column groups of 32 pair-rows each per core
F16 = mybir.dt.float16
F32 = mybir.dt.float32

NCOL = P + G * B   # stationary W cols + 4096 payload cols
MMN = 512          # moving columns per matmul (one PSUM bank)
NWARM = 13         # dummy matmuls to warm the PE clock gate


def _build_nc() -> bacc.Bacc:
    """Build the per-core Bass/Tile program (identical on all 8 cores)."""
    nc = bacc.Bacc("TRN2", enable_partition_id=False)

    xb = nc.dram_tensor("xb", [P, NCOL], F16, kind="ExternalInput")
    yb = nc.dram_tensor("yb", [P, G * B], F16, kind="ExternalOutput")

    with TileContext(nc) as tc:
        with (
            tc.tile_pool(name="warm", bufs=1) as warm_pool,
            tc.tile_pool(name="io", bufs=1) as io_pool,
            tc.tile_pool(name="psum", bufs=4, space="PSUM") as psum_pool,
            tc.tile_pool(name="psum_w", bufs=1, space="PSUM") as psum_w_pool,
        ):
            # PE warmup: matmuls over a zeroed scratch keep the PE busy
            # through the load window so the HAM clock gate reaches 8/8
            # before the payload matmuls.
            wz = warm_pool.tile([P, 256], F16)
            nc.gpsimd.memset(wz[:], 0)
            wp = psum_w_pool.tile([P, 256], F32)
            for _ in range(NWARM):
                nc.tensor.matmul(wp[:], wz[:, 0:P], wz[:],
                                 start=True, stop=True)

            xb_sb = io_pool.tile([P, NCOL], F16, name="xb_sb")
            yb_sb = io_pool.tile([P, G * B], F16, name="yb_sb")

            # loads on sync: small first chunk (W + one matmul's columns)
            # so compute starts early; larger-rowed chunks after (wider
            # contiguous rows -> bigger DMA packets -> better per-engine
            # rate).  Only 3 load sems: bacc's kernel sem pool is 12 and
            # exhausting it forces serializing sem-reuse waits.
            load_edges = [0, P + 512, P + 2560, NCOL]
            for a, b in zip(load_edges, load_edges[1:]):
                nc.sync.dma_start(xb_sb[:, a:b], xb[:, a:b])

            # store chunks (payload columns): wide until the tail, where
            # each store waits on as little compute as possible.
            store_edges = [0, 1536, 3072, 3584, 4096]
            si = 0

            w_sb = xb_sb[:, 0:P]
            for g in range(G):
                for h in range(2):
                    ci = g * B + h * MMN
                    pt = psum_pool.tile([P, MMN], F32, tag="ps")
                    nc.tensor.matmul(pt[:], w_sb,
                                     xb_sb[:, P + ci : P + ci + MMN],
                                     start=True, stop=True)
                    # alternate the PSUM drain across DVE / ACT; the final
                    # slab splits across both so the tail cast is half-size
                    if g == G - 1 and h == 1:
                        hn = MMN // 2
                        nc.vector.tensor_copy(yb_sb[:, ci : ci + hn],
                                              pt[:, 0:hn])
                        nc.scalar.copy(yb_sb[:, ci + hn : ci + MMN],
                                       pt[:, hn:MMN])
                    elif h == 0:
                        nc.vector.tensor_copy(yb_sb[:, ci : ci + MMN], pt[:])
                    else:
                        nc.scalar.copy(yb_sb[:, ci : ci + MMN], pt[:])
                    # stores ride the sync ring FIFO behind the loads, so
                    # loads are never blocked
                    while si < 4 and store_edges[si + 1] <= ci + MMN:
                        ss = slice(store_edges[si], store_edges[si + 1])
                        nc.sync.dma_start(yb[:, ss], yb_sb[:, ss])
                        si += 1

    nc.finalize()
    return nc


_NC_CACHE = None


def _get_nc() -> bacc.Bacc:
    global _NC_CACHE
    if _NC_CACHE is None:
        _NC_CACHE = _build_nc()
    return _NC_CACHE


def _mix_matrix(M_re: np.ndarray, M_im: np.ndarray) -> np.ndarray:
    """Host-side 2x2 expm of the anti-Hermitian generator -> real 4x4 C.

    C rows are output quantities (o1re, o1im, o2re, o2im); columns are
    input kinds (x1re, x1im, x2re, x2im).
    """
    M = M_re.astype(np.float64) + 1j * M_im.astype(np.float64)
    A = M - M.conj().T          # anti-Hermitian
    H = -1j * A                 # Hermitian
    w, V = np.linalg.eigh(H)
    Mexp = V @ np.diag(np.exp(1j * w)) @ V.conj().T   # expm(A), exact
    c00, c01, c10, c11 = Mexp[0, 0], Mexp[0, 1], Mexp[1, 0], Mexp[1, 1]
    C = np.array([
        [c00.real, -c00.imag, c01.real, -c01.imag],
        [c00.imag,  c00.real, c01.imag,  c01.real],
        [c10.real, -c10.imag, c11.real, -c11.imag],
        [c10.imag,  c10.real, c11.imag,  c11.real],
    ], dtype=np.float64)
    return C


def _pack_inputs(M_re, M_im, x_re, x_im):
    """Build per-core input maps: packed fp16 [W | xb] tiles."""
    C = _mix_matrix(M_re, M_im)
    W = np.kron(C.T, np.eye(32)).astype(np.float16)   # [128, 128]

    # [d, kind, g, r, c] -> [d, kind*32+r, g*1024+c]
    x1r = x_re[2048:3072].reshape(NCORES, G, 32, B)
    x1i = x_im[2048:3072].reshape(NCORES, G, 32, B)
    x2r = x_re[3072:4096].reshape(NCORES, G, 32, B)
    x2i = x_im[3072:4096].reshape(NCORES, G, 32, B)
    arr = np.stack([x1r, x1i, x2r, x2i], axis=1)      # [d, kind, g, r, c]
    arr = arr.transpose(0, 1, 3, 2, 4)                # [d, kind, r, g, c]

    xb_all = np.empty((NCORES, P, NCOL), dtype=np.float16)
    xb_all[:, :, 0:P] = W[None]
    xb_all[:, :, P:] = arr.reshape(NCORES, P, G * B)
    return [{"xb": xb_all[d]} for d in range(NCORES)]


def _unpack_output(x_re, x_im, results) -> np.ndarray:
    """Assemble the full complex64 output from fp16 device results."""
    yb_all = np.stack([r["yb"] for r in results])     # [d, 128, 4096] f16
    # [d, q, r, g, c] -> quantity q at bottom row d*128 + g*32 + r
    q = yb_all.reshape(NCORES, G, 32, G, B).transpose(1, 0, 3, 2, 4)
    q = q.reshape(G, B, B).astype(np.float32)         # [q, 1024 rows, 1024]

    full = np.empty((D, 2 * B), dtype=np.float32)
    full[:2048, 0::2] = x_re[:2048]
    full[:2048, 1::2] = x_im[:2048]
    full[2048:3072, 0::2] = q[0]
    full[2048:3072, 1::2] = q[1]
    full[3072:4096, 0::2] = q[2]
    full[3072:4096, 1::2] = q[3]
    return full.view(np.complex64)  # (4096, 1024)


def kernel(M_re, M_im, x_re, x_im) -> np.ndarray:
    M_re = np.asarray(M_re, dtype=np.float32)
    M_im = np.asarray(M_im, dtype=np.float32)
    x_re = np.ascontiguousarray(x_re, dtype=np.float32)
    x_im = np.ascontiguousarray(x_im, dtype=np.float32)

    in_maps = _pack_inputs(M_re, M_im, x_re, x_im)
    nc = _get_nc()
    res = run_bass_kernel_spmd(nc, in_maps, core_ids=list(range(NCORES)))
    return _unpack_output(x_re, x_im, res.results)
